# revision 26
# baseline (speedup 1.0000x reference)
"""Trainium2 Bass kernel for a pre-norm transformer block (E=512, H=2048, NH=8, N=4096).

Sharding: sequence-parallel over 8 NeuronCores with per-core token ROLL: core c
receives x with tokens rotated so its own 512-token slice is chunk 0. Each core
computes the full K/V projection (all 4096 tokens, fp8 DoubleRow matmuls) but
only its own slice of queries, attention output, MLP and residuals. No
collectives; host concatenates slices.

Precision strategy (validated numerically, rel-err budget 2e-2):
  - K/V projection runs in fp8e4m3 DoubleRow (2 k-tiles per matmul, 2x PE
    throughput). LayerNorm'd activations are quantized to fp8 on the fly.
  - Q projection, scores, PV, proj and the MLP stay bf16.
  - LayerNorm gain/bias are folded into qkv/fc1 weights host-side, so the
    on-chip LN apply is (x - mu) * rstd only (2 DVE ops).
  - Residual stream is fp32 end to end.
"""
import sys

sys.path.insert(0, "/opt/trn_rl_repo")
sys.path.insert(0, "/opt/pypackages")

import numpy as np

E, H, NH, HD = 512, 2048, 8, 64
T, NCORES = 4096, 8
TC = T // NCORES          # tokens per core
P = 128
ET = E // P               # 4  feature tiles of E
HT = H // P               # 16 feature tiles of H
KT = T // P               # 32 key-token tiles
CW = 512                  # token chunk width in phase 1
NCH = T // CW             # 8  token chunks
EPS = 1e-5

_BUILT = None


def _build():
    import concourse.bacc as bacc
    import concourse.mybir as mybir
    import concourse.tile as tile

    dt = mybir.dt
    F32 = dt.float32
    BF16 = dt.bfloat16
    F8 = dt.float8e4

    nc = bacc.Bacc("TRN2", target_bir_lowering=False, debug=False, num_devices=NCORES)

    d = {}
    d["d_xT"] = nc.dram_tensor("xT", [E, T], BF16, kind="ExternalInput").ap()
    d["d_xsT"] = nc.dram_tensor("xsT", [E, TC], F32, kind="ExternalInput").ap()
    d["d_wqT"] = nc.dram_tensor("wqT", [E, E], BF16, kind="ExternalInput").ap()
    d["d_wkvT8"] = nc.dram_tensor("wkvT8", [E, 2 * E], F8, kind="ExternalInput").ap()
    d["d_bqkv"] = nc.dram_tensor("bqkv", [3 * E], F32, kind="ExternalInput").ap()
    d["d_wprojT"] = nc.dram_tensor("wprojT", [E, E], BF16, kind="ExternalInput").ap()
    d["d_bproj"] = nc.dram_tensor("bproj", [E], F32, kind="ExternalInput").ap()
    d["d_wfc1T"] = nc.dram_tensor("wfc1T", [E, H], BF16, kind="ExternalInput").ap()
    d["d_bfc1"] = nc.dram_tensor("bfc1", [H], F32, kind="ExternalInput").ap()
    d["d_wfc2T"] = nc.dram_tensor("wfc2T", [H, H], BF16, kind="ExternalInput").ap()
    d["d_bfc2"] = nc.dram_tensor("bfc2", [H], F32, kind="ExternalInput").ap()
    d["d_wfc3T"] = nc.dram_tensor("wfc3T", [H, E], BF16, kind="ExternalInput").ap()
    d["d_bfc3"] = nc.dram_tensor("bfc3", [E], F32, kind="ExternalInput").ap()
    d["d_outT"] = nc.dram_tensor("outT", [E, TC], F32, kind="ExternalOutput").ap()

    with tile.TileContext(nc) as tc:
        _emit(nc, tc, tile, mybir, d)

    nc.compile()
    return nc


def _emit(nc, tc, tile, mybir, d):
    dt = mybir.dt
    AF = mybir.ActivationFunctionType
    OP = mybir.AluOpType
    PM = mybir.MatmulPerfMode.DoubleRow
    F32 = dt.float32
    BF16 = dt.bfloat16
    F8 = dt.float8e4

    def pool(**kw):
        p = tc.tile_pool(**kw)
        return p.__enter__(), p

    def close(*ps):
        for p in ps:
            p.__exit__(None, None, None)

    # ---- long-lived pools ----
    consts, _c0 = pool(name="consts", bufs=1, side="left")
    stats, _c1 = pool(name="stats", bufs=2, side="left")
    bcast, _c2 = pool(name="bcast", bufs=2, side="left")
    scratch, _c3 = pool(name="scratch", bufs=6, side="left")
    STAT = [pool(name="ps_stat", bufs=2, space="PSUM")]
    drp, _cd = pool(name="drscratch", bufs=4, space="DRAM")

    # ---- constants ----
    onesf = consts.tile([P, 1], F32)
    nc.vector.memset(onesf[:], 1.0)
    ones_w = consts.tile([P, 1], BF16)
    nc.vector.tensor_copy(ones_w[:], onesf[:])
    eps_t = consts.tile([1, 1], F32)
    nc.vector.memset(eps_t[:], EPS)
    eps_p = consts.tile([P, 1], F32)
    nc.vector.memset(eps_p[:], EPS)

    def ld_vec(dram, n, name):  # [n] f32 -> [P, n//P] per-partition layout
        t = consts.tile([P, n // P], F32, name=name)
        nc.sync.dma_start(t[:], dram.rearrange("(m p) -> p m", p=P))
        return t

    bq_sb = ld_vec(d["d_bqkv"][0:E], E, "bq_sb")
    bk_sb = ld_vec(d["d_bqkv"][E:2 * E], E, "bk_sb")
    bv_sb = ld_vec(d["d_bqkv"][2 * E:3 * E], E, "bv_sb")
    bproj_sb = ld_vec(d["d_bproj"], E, "bproj_sb")
    bfc1_sb = ld_vec(d["d_bfc1"], H, "bfc1_sb")
    bfc2_sb = ld_vec(d["d_bfc2"], H, "bfc2_sb")
    bfc3_sb = ld_vec(d["d_bfc3"], E, "bfc3_sb")

    def recip_bcast(dst_bb, src_1w, w, rsqrt=False):
        """dst_bb[P, w] = broadcast(1/src) or broadcast(1/sqrt(src + eps)).

        Single-partition DVE reciprocal is slow, so spread the w values over
        128 partitions via a DRAM bounce, invert there, then broadcast back
        with a stride-0 DMA read.
        """
        dr1 = drp.tile([w], F32, tag="dr1", name="dr1")
        nc.sync.dma_start(dr1[None, :], src_1w)
        pk = scratch.tile([P, w // P], F32, tag="rpk", name="rpk")
        nc.sync.dma_start(pk[:], dr1.rearrange("(p f) -> p f", p=P))
        if rsqrt:
            nc.scalar.activation(pk[:], pk[:], AF.Sqrt, bias=eps_p[:])
        nc.vector.reciprocal(pk[:], pk[:])
        dr2 = drp.tile([w], F32, tag="dr2", name="dr2")
        nc.sync.dma_start(dr2.rearrange("(p f) -> p f", p=P), pk[:])
        src_b = dr2[None, :].to_broadcast((P, w))
        if dst_bb.dtype == F32:
            nc.sync.dma_start(dst_bb, src_b)
        else:
            nc.gpsimd.dma_start(dst_bb, src_b)

    def bcast_dram(dst_bb, src_1w, w):
        """dst_bb[P, w] = broadcast(src[1, w]) via DRAM stride-0 read."""
        dr = drp.tile([w], F32, tag="drb", name="drb")
        nc.sync.dma_start(dr[None, :], src_1w)
        src_b = dr[None, :].to_broadcast((P, w))
        if dst_bb.dtype == F32:
            nc.sync.dma_start(dst_bb, src_b)
        else:
            nc.gpsimd.dma_start(dst_bb, src_b)

    def ln_chain(mu_ps, sq_ps, w):
        """mu/sq accumulators -> broadcast (mu_b, rs_b) [P, w] bf16 tiles.

        ACT sqrt + DVE fast reciprocal + gpsimd partition_broadcast: no DRAM
        bounce, ~2.5us latency.
        """
        mu = stats.tile([1, w], F32, tag="mu_sb", name="mu_sb")
        ms = stats.tile([1, w], F32, tag="ms_sb", name="ms_sb")
        nc.scalar.mul(mu[:], mu_ps[:], 1.0 / E)
        nc.scalar.mul(ms[:], sq_ps[:], 1.0 / E)
        var = stats.tile([1, w], F32, tag="var", name="var")
        nc.vector.tensor_mul(var[:], mu[:], mu[:])
        nc.vector.tensor_sub(var[:], ms[:], var[:])
        rstd = stats.tile([1, w], F32, tag="rstd_f", name="rstd_f")
        nc.scalar.activation(rstd[:], var[:], AF.Sqrt, bias=eps_t[:])
        nc.vector.reciprocal_approx_fast(rstd[:], rstd[:])
        mu_w = stats.tile([1, w], BF16, tag="mu_w", name="mu_w")
        rs_w = stats.tile([1, w], BF16, tag="rs_w", name="rs_w")
        nc.vector.tensor_copy(mu_w[:], mu[:])
        nc.vector.tensor_copy(rs_w[:], rstd[:])
        mu_b = bcast.tile([P, w], BF16, tag="mu_b", name="mu_b")
        rs_b = bcast.tile([P, w], BF16, tag="rs_b", name="rs_b")
        nc.gpsimd.partition_broadcast(mu_b[:], mu_w[:])
        nc.gpsimd.partition_broadcast(rs_b[:], rs_w[:])
        return mu_b, rs_b

    # ---- K/V/Q tensors live until end of attention ----
    big, h_big = pool(name="big", bufs=1, side="right")
    KTs = big.tile([P, ET, T], BF16)            # K^T, feature-major
    V65 = big.tile([P, KT, NH, HD + 1], BF16)   # V token-major + ones col
    QTs = big.tile([P, ET, TC], BF16)

    # ones column of V65 (denominator rides the PV matmul)
    nc.vector.tensor_copy(
        V65[:, :, :, HD:HD + 1],
        onesf[:, :, None, None].to_broadcast((P, KT, NH, 1)))

    # ====== phase 1: stream x in 512-token chunks, LN, fp8 K/V projection.
    # Software-pipelined: chunk n+1's LN statistics run on the PE before
    # chunk n's projections so the LN chain latency is hidden. ======
    ps_mm4, h_ps_mm4 = pool(name="ps_mm4", bufs=4, space="PSUM")
    wqp, h_wqp = pool(name="wq", bufs=1, side="right")
    wq = wqp.tile([P, ET, E], BF16)
    wkvp, h_wkvp = pool(name="wkv", bufs=1, side="right")
    wkv8 = wkvp.tile([P, ET, 2 * E], F8)
    xcp, h_xcp = pool(name="xc", bufs=3, side="right")
    lnp, h_lnp = pool(name="lnp", bufs=2, side="right")

    persistA, h_persistA = pool(name="persistA", bufs=1, side="left")
    xs_sb = persistA.tile([P, ET, TC], F32)
    UTs = persistA.tile([P, ET, TC], BF16)      # attention output

    def warmup(pool_, n, rhs):
        wps = pool_.tile([1, rhs.shape[-1]], F32, tag="mm", name="wps")
        for i in range(n):
            nc.tensor.matmul(wps[:], ones_w[:], rhs,
                             start=(i == 0), stop=(i == n - 1),
                             skip_group_check=True)

    def stats_emit(xx):
        mu_ps = STAT[0][0].tile([1, CW], F32, tag="mu")
        sq_ps = STAT[0][0].tile([1, CW], F32, tag="sq")
        for e in range(ET):
            nc.tensor.matmul(mu_ps[:], ones_w[:], xx[:, e, 0, :],
                             start=(e == 0), stop=(e == ET - 1))
        for e in range(ET):
            nc.tensor.matmul(sq_ps[:], ones_w[:], xx[:, e, 1, :],
                             start=(e == 0), stop=(e == ET - 1))
        return mu_ps, sq_ps

    def kv_project(st):
        xx, ln8, mu_b, rs_b, ch = st
        # LN apply -> fp8 (g/b folded into weights host-side)
        tmp = xcp.tile([P, ET, CW], BF16, tag="lntmp", name="lntmp")
        for e in range(ET):
            nc.vector.tensor_sub(tmp[:, e, :], xx[:, e, 0, :], mu_b[:])
            nc.vector.tensor_mul(ln8[:, e, :], tmp[:, e, :], rs_b[:])
        if ch == 0:
            # bf16 LN of own slice feeds the (bf16) Q projection
            lnq = xcp.tile([P, ET, CW], BF16, tag="lnq", name="lnq")
            for e in range(ET):
                nc.vector.tensor_mul(lnq[:, e, :], tmp[:, e, :], rs_b[:])
        # K projection (fp8 DoubleRow, 2 e-tile pairs)
        for m in range(ET):
            kps = ps_mm4.tile([P, CW], F32, tag="mm", name="kps")
            for j in range(ET // 2):
                nc.tensor.matmul(kps[:],
                                 wkv8[:, 2 * j:2 * j + 2, m * P:(m + 1) * P],
                                 ln8[:, 2 * j:2 * j + 2, :],
                                 start=(j == 0), stop=(j == ET // 2 - 1),
                                 perf_mode=PM)
            nc.scalar.activation(KTs[:, m, ch * CW:(ch + 1) * CW], kps[:],
                                 AF.Identity, bias=bk_sb[:, m:m + 1])
        # V projection (fp8 DoubleRow, token-major output)
        for t4 in range(CW // P):
            vps = ps_mm4.tile([P, E], F32, tag="mm", name="vps")
            for j in range(ET // 2):
                nc.tensor.matmul(vps[:],
                                 ln8[:, 2 * j:2 * j + 2, t4 * P:(t4 + 1) * P],
                                 wkv8[:, 2 * j:2 * j + 2, E:2 * E],
                                 start=(j == 0), stop=(j == ET // 2 - 1),
                                 perf_mode=PM)
            kt = ch * (CW // P) + t4
            nc.scalar.activation(
                V65[:, kt, :, 0:HD],
                vps[:].rearrange("p (h d) -> p h d", h=NH), AF.Copy)
        # Q projection (bf16) for chunk 0 == own token slice
        if ch == 0:
            for m in range(ET):
                qps = ps_mm4.tile([P, CW], F32, tag="mm", name="qps")
                for e in range(ET):
                    nc.tensor.matmul(qps[:], wq[:, e, m * P:(m + 1) * P],
                                     lnq[:, e, :],
                                     start=(e == 0), stop=(e == ET - 1))
                nc.scalar.activation(QTs[:, m, :], qps[:], AF.Identity,
                                     bias=bq_sb[:, m:m + 1])

    pending = []
    for ch in range(NCH):
        xx = xcp.tile([P, ET, 2, CW], BF16, tag="xc", name="xc")
        nc.sync.dma_start(
            xx[:, :, 0, :],
            d["d_xT"][:, ch * CW:(ch + 1) * CW].rearrange("(m p) t -> p m t", p=P))
        if ch == 0:
            warmup(ps_mm4, 12, xx[:, 0, 0, :])  # bridge the LN latency at t=0
            nc.sync.dma_start(
                wkv8[:], d["d_wkvT8"].rearrange("(m p) o -> p m o", p=P))
            nc.sync.dma_start(
                wq[:], d["d_wqT"].rearrange("(m p) o -> p m o", p=P))
            nc.sync.dma_start(
                xs_sb[:], d["d_xsT"].rearrange("(m p) t -> p m t", p=P))
        for e in range(ET):
            nc.vector.tensor_mul(xx[:, e, 1, :], xx[:, e, 0, :], xx[:, e, 0, :])
        mu_ps, sq_ps = stats_emit(xx)
        mu_b, rs_b = ln_chain(mu_ps, sq_ps, CW)
        ln8 = lnp.tile([P, ET, CW], F8, tag="ln8", name="ln8")
        pending.append((xx, ln8, mu_b, rs_b, ch))
        if len(pending) == 2:
            kv_project(pending.pop(0))
    while pending:
        kv_project(pending.pop(0))
    warmup(ps_mm4, 16, KTs[:, 0, 0:TC])         # bridge K-chain -> attention
    close(h_lnp, h_xcp, h_wkvp, h_wqp)
    close(h_ps_mm4)
    close(STAT[0][1])

    # ====== phase 3: attention (2 heads in flight, exp over kt-pairs) ======
    ps_sc, h_ps_sc = pool(name="ps_sc", bufs=3, space="PSUM")
    ps_pv, h_ps_pv = pool(name="ps_pv", bufs=1, space="PSUM")
    ptp, h_ptp = pool(name="ptile", bufs=4, side="right")
    stp, h_stp = pool(name="stage", bufs=2, side="right")
    scale = float(HD) ** -0.5
    for mp in range(ET):
        heads = [2 * mp, 2 * mp + 1]
        pvs = [ps_pv.tile([HD + 1, TC], F32, tag=f"pv{j}", name="pv")
               for j in range(2)]

        def emit_pv(k0, pts):
            for j, h in enumerate(heads):
                nc.tensor.matmul(pvs[j][:], V65[:, k0, h, :], pts[j][:, 0:TC],
                                 start=(k0 == 0), stop=False,
                                 skip_group_check=True)
                nc.tensor.matmul(pvs[j][:], V65[:, k0 + 1, h, :],
                                 pts[j][:, TC:2 * TC],
                                 start=False, stop=(k0 + 1 == KT - 1),
                                 skip_group_check=True)

        # PV runs one kt-pair behind scores so the PE never waits on exp.
        lag = None
        for ktp in range(KT // 2):
            k0 = 2 * ktp
            pts = []
            for j, h in enumerate(heads):
                lo = (h % 2) * HD
                m = h // 2
                sc2 = ps_sc.tile([P, 2 * TC], F32, tag="sc2", name="sc2")
                nc.tensor.matmul(sc2[:, 0:TC],
                                 KTs[lo:lo + HD, m, k0 * P:(k0 + 1) * P],
                                 QTs[lo:lo + HD, m, :], skip_group_check=True)
                nc.tensor.matmul(sc2[:, TC:2 * TC],
                                 KTs[lo:lo + HD, m, (k0 + 1) * P:(k0 + 2) * P],
                                 QTs[lo:lo + HD, m, :], skip_group_check=True)
                pt2 = ptp.tile([P, 2 * TC], BF16, tag="pt2", name="pt2")
                nc.scalar.activation(pt2[:], sc2[:], AF.Exp, scale=scale)
                pts.append(pt2)
            if lag is not None:
                emit_pv(*lag)
            lag = (k0, pts)
        emit_pv(*lag)
        for j, h in enumerate(heads):
            lo = (h % 2) * HD
            m = h // 2
            pv = pvs[j]
            stg = stp.tile([HD, TC], BF16, tag="stg", name="stg")
            nc.vector.tensor_copy(stg[:], pv[0:HD, :])
            # denominator row sits on psum partition 64; engines cannot shift
            # partitions and DMA cannot read PSUM, so: copy at offset 64 to
            # SBUF, then DMA-shift to partition 0 for broadcast.
            den65 = stp.tile([HD + 1, TC], F32, tag="stg_s", name="stg_s")
            nc.vector.tensor_copy(den65[HD:HD + 1, :], pv[HD:HD + 1, :])
            den = stats.tile([1, TC], F32, tag="den", name="den")
            nc.sync.dma_start(den[:], den65[HD:HD + 1, :])
            nc.sync.dma_start(UTs[lo:lo + HD, m, :], stg[:])
            nc.vector.reciprocal_approx_fast(den[:], den[:])
            rb = bcast.tile([P, TC], F32, tag="rb", name="rb")
            nc.gpsimd.partition_broadcast(rb[:], den[:])
            nc.vector.tensor_mul(UTs[lo:lo + HD, m, :], UTs[lo:lo + HD, m, :],
                                 rb[lo:lo + HD, :])
            nc.vector.tensor_scalar_add(UTs[lo:lo + HD, m, :],
                                        UTs[lo:lo + HD, m, :],
                                        scalar1=bv_sb[lo:lo + HD, m:m + 1])
    close(h_stp, h_ptp, h_ps_pv, h_ps_sc)
    close(h_big)                     # K/V/Q dead after attention

    # ============ phase 4: output proj + residual + LN2 ============
    STAT[0] = pool(name="ps_stat2", bufs=2, space="PSUM")
    ps_mm, h_ps_mm = pool(name="ps_mm", bufs=4, space="PSUM")
    persistB, h_persistB = pool(name="persistB", bufs=1, side="left")
    x1_sb = persistB.tile([P, ET, TC], F32)
    h2_sb = persistB.tile([P, ET, TC], BF16)
    outsb = persistB.tile([P, ET, TC], F32)
    wpp, h_wpp = pool(name="wproj", bufs=1, side="left")
    wproj = wpp.tile([P, ET, E], BF16)
    nc.sync.dma_start(wproj[:], d["d_wprojT"].rearrange("(m p) o -> p m o", p=P))

    # e-major proj: partials for head-pair e are emitted as soon as that
    # pair's attention output exists, so the PE streams through the
    # attention tail instead of idling on the last pair's denominators.
    pps = [ps_mm.tile([P, TC], F32, tag="mm", name="pps") for _ in range(ET)]
    for e in range(ET):
        for m in range(ET):
            nc.tensor.matmul(pps[m][:], wproj[:, e, m * P:(m + 1) * P],
                             UTs[:, e, :], start=(e == 0), stop=(e == ET - 1),
                             skip_group_check=True)
    mu2_ps = STAT[0][0].tile([1, TC], F32, tag="mu")
    sq2_ps = STAT[0][0].tile([1, TC], F32, tag="sq")
    xw_tiles = []
    for m in range(ET):
        # x1 = (proj + bias) + x_slice
        nc.vector.scalar_tensor_tensor(
            x1_sb[:, m, :], pps[m][:], bproj_sb[:, m:m + 1], xs_sb[:, m, :],
            op0=OP.add, op1=OP.add)
        # LN2 statistics accumulate as each x1 block lands
        xx2 = scratch.tile([P, 2, TC], BF16, tag="ln_xw", name="ln_xw")
        nc.vector.tensor_copy(xx2[:, 0, :], x1_sb[:, m, :])
        nc.vector.tensor_mul(xx2[:, 1, :], xx2[:, 0, :], xx2[:, 0, :])
        nc.tensor.matmul(mu2_ps[:], ones_w[:], xx2[:, 0, :],
                         start=(m == 0), stop=(m == ET - 1), skip_group_check=True)
        nc.tensor.matmul(sq2_ps[:], ones_w[:], xx2[:, 1, :],
                         start=(m == 0), stop=(m == ET - 1), skip_group_check=True)
        xw_tiles.append(xx2)
    close(h_wpp)
    mu_b2, rs_b2 = ln_chain(mu2_ps, sq2_ps, TC)
    for m in range(ET):
        nc.vector.tensor_sub(h2_sb[:, m, :], xw_tiles[m][:, 0, :], mu_b2[:])
        nc.vector.tensor_mul(h2_sb[:, m, :], h2_sb[:, m, :], rs_b2[:])

    # ============ phase 5: MLP ============
    mlp, h_mlp = pool(name="mlp", bufs=1, side="left")
    m1_sb = mlp.tile([P, HT, TC], BF16)
    m2_sb = mlp.tile([P, HT, TC], BF16)
    w1p, h_w1p = pool(name="wfc1", bufs=1, side="left")
    wfc1 = w1p.tile([P, ET, H], BF16)
    nc.sync.dma_start(wfc1[:], d["d_wfc1T"].rearrange("(m p) o -> p m o", p=P))
    w3p, h_w3p = pool(name="wfc3", bufs=1, side="left")
    wfc3 = w3p.tile([P, HT, E], BF16)
    nc.sync.dma_start(wfc3[:], d["d_wfc3T"].rearrange("(m p) o -> p m o", p=P))
    w2p, h_w2p = pool(name="wfc2c", bufs=1, side="left")
    wcs = []
    for e in range(HT):
        wc = w2p.tile([P, H], BF16, tag=f"wc{e}", name="wc")
        nc.sync.dma_start(wc[:], d["d_wfc2T"][e * P:(e + 1) * P, :])
        wcs.append(wc)
    warmup(ps_mm, 12, wfc1[:, 0, 0:TC])         # bridge LN2 chain -> fc1
    for m in range(HT):
        ps1 = ps_mm.tile([P, TC], F32, tag="mm", name="ps1")
        for e in range(ET):
            nc.tensor.matmul(ps1[:], wfc1[:, e, m * P:(m + 1) * P],
                             h2_sb[:, e, :], start=(e == 0), stop=(e == ET - 1))
        nc.scalar.activation(m1_sb[:, m, :], ps1[:], AF.Relu,
                             bias=bfc1_sb[:, m:m + 1])
    close(h_ps_mm, STAT[0][1])

    # fc2 with fc3 partials interleaved: as each m2 block lands, its fc3
    # contribution accumulates, so fc3 finishes with fc2 instead of after it.
    ps_f3, h_ps_f3 = pool(name="ps_f3", bufs=4, space="PSUM")
    ps8p, h_ps8p = pool(name="ps8", bufs=4, space="PSUM")
    ps3s = [ps_f3.tile([P, TC], F32, tag="f3", name="ps3") for _ in range(ET)]
    for m in range(HT):
        psm = ps8p.tile([P, TC], F32, tag="mm8", name="psm")
        for e in range(HT):
            nc.tensor.matmul(psm[:], wcs[e][:, m * P:(m + 1) * P],
                             m1_sb[:, e, :],
                             start=(e == 0), stop=(e == HT - 1),
                             skip_group_check=True)
        nc.scalar.activation(m2_sb[:, m, :], psm[:], AF.Relu,
                             bias=bfc2_sb[:, m:m + 1])
        for mo in range(ET):
            nc.tensor.matmul(ps3s[mo][:], wfc3[:, m, mo * P:(mo + 1) * P],
                             m2_sb[:, m, :], start=(m == 0), stop=(m == HT - 1),
                             skip_group_check=True)
    close(h_ps8p, h_w2p)
    for m in range(ET):
        nc.vector.scalar_tensor_tensor(
            outsb[:, m, :], ps3s[m][:], bfc3_sb[:, m:m + 1], x1_sb[:, m, :],
            op0=OP.add, op1=OP.add)
        nc.sync.dma_start(d["d_outT"][m * P:(m + 1) * P, :], outsb[:, m, :])
    close(h_ps_f3, h_w3p, h_w1p, h_mlp, h_persistB, h_persistA)
    close(_cd)
    close(_c3, _c2, _c1, _c0)


def _get_nc():
    global _BUILT
    if _BUILT is None:
        _BUILT = _build()
    return _BUILT


def run(inputs, trace=False):
    from concourse.bass_utils import run_bass_kernel_spmd

    nc = _get_nc()
    import ml_dtypes
    bf16 = ml_dtypes.bfloat16
    f8 = ml_dtypes.float8_e4m3
    x = np.asarray(inputs["x"], np.float32)[0]          # [T, E]
    ln_g = np.asarray(inputs["ln_g"], np.float32)
    ln_b = np.asarray(inputs["ln_b"], np.float32)
    qkv_w = np.asarray(inputs["qkv_w"], np.float32)
    qkv_b = np.asarray(inputs["qkv_b"], np.float32)
    fc1_w = np.asarray(inputs["fc1_w"], np.float32)
    fc1_b = np.asarray(inputs["fc1_b"], np.float32)

    # fold LN affine into the first-layer weights of each sublayer
    qkv_wf = qkv_w * ln_g[None, :]
    qkv_bf = qkv_b + qkv_w @ ln_b
    fc1_wf = fc1_w * ln_g[None, :]
    fc1_bf = fc1_b + fc1_w @ ln_b

    ct = lambda a: np.ascontiguousarray(np.asarray(a, np.float32).T)
    common = {
        "wqT": ct(qkv_wf[0:E]).astype(bf16),
        "wkvT8": ct(qkv_wf[E:3 * E]).astype(f8),
        "bqkv": qkv_bf,
        "wprojT": ct(inputs["proj_w"]).astype(bf16),
        "bproj": np.asarray(inputs["proj_b"], np.float32),
        "wfc1T": ct(fc1_wf).astype(bf16),
        "bfc1": fc1_bf,
        "wfc2T": ct(inputs["fc2_w"]).astype(bf16),
        "bfc2": np.asarray(inputs["fc2_b"], np.float32),
        "wfc3T": ct(inputs["fc3_w"]).astype(bf16),
        "bfc3": np.asarray(inputs["fc3_b"], np.float32),
    }
    in_maps = []
    for c in range(NCORES):
        xr = np.roll(x, -c * TC, axis=0)                # own slice first
        in_maps.append({
            **common,
            "xT": ct(xr).astype(bf16),
            "xsT": ct(x[c * TC:(c + 1) * TC, :]),
        })
    res = run_bass_kernel_spmd(nc, in_maps, core_ids=list(range(NCORES)),
                               trace=trace)
    out = np.empty((1, T, E), np.float32)
    for c in range(NCORES):
        out[0, c * TC:(c + 1) * TC, :] = res.results[c]["outT"].T
    return out, res


def kernel(**inputs) -> np.ndarray:
    out, _ = run(inputs, trace=False)
    return out
